# revision 1
# baseline (speedup 1.0000x reference)
"""AdaptiveGraphLearner distributed Trainium2 kernel (8 NeuronCores), v3.

reference:  sim = (x @ x.T)/0.1;  adj = sim * rowwise_top32_mask(sim)
            out = (adj + adj.T)/2

Math identical to the working baseline (row-sharded; per-row e32/e33
thresholds; 4KB AllGather of column-threshold midpoints instead of a 32MB
adj transpose), restructured for engine balance:

- Phase 1 extracts per-row thresholds straight from PSUM with DVE max8
  (raw x@x.T values, no scaling anywhere): cross-core bitwise symmetry
  h[a,b]==h[b,a] of fp32r matmuls holds exactly as in the baseline.
- PSUM is organized as [128,2048] 4-bank mega-tiles (2 in flight).
- Phase 3 per mega-tile: ScalarE emits h5 = 5*psum as bf16 (output values
  only), DVE does the two compares on RAW psum (column compare vs cb,
  fused row-compare+add STT), and Pool (which only implements add/mult)
  does the final h5*mask multiply into the bf16 output tile.
- Output is bf16; host upcasts to fp32.
"""
import sys
sys.path.insert(0, '/opt/trn_rl_repo')
import numpy as np
import concourse.bass as bass
import concourse.bacc as bacc
import concourse.mybir as mybir
import concourse.tile as tile
from concourse.bass_utils import run_bass_kernel_spmd

N, DIM, K = 8192, 256, 32
TEMP = 0.1
SCALE = 0.5 / TEMP
NCORES = 8
RPC = N // NCORES          # 1024 rows per core
NB = RPC // 128            # 8 row-blocks of 128
NMEGA = N // 2048          # 4 mega-tiles of 2048 per block
NEG = -1e30
SIGBIG = 1.0e6

f32 = mybir.dt.float32
f32r = mybir.dt.float32r
bf16 = mybir.dt.bfloat16
COPY = mybir.ActivationFunctionType.Copy
SIG = mybir.ActivationFunctionType.Sigmoid
GT = mybir.AluOpType.is_gt
ADD = mybir.AluOpType.add
MUL = mybir.AluOpType.mult



def build_nc():
    nc = bacc.Bacc(None, target_bir_lowering=False, num_devices=NCORES)
    xT = nc.declare_dram_parameter("xT", [DIM, N], f32r, isOutput=False)
    xgT = nc.declare_dram_parameter("xgT", [DIM, RPC], f32r, isOutput=False)
    out = nc.declare_dram_parameter("out", [RPC, N], bf16, isOutput=True)

    with tile.TileContext(nc) as tc:
        with tc.tile_pool(name="dram", bufs=1, space="DRAM") as dram:
            t_loc_a = dram.tile([7 * 128], f32)
            t_loc_b = dram.tile([128], f32)
            t_all_a = dram.tile([NCORES * 7 * 128], f32, addr_space="Shared")
            t_all_b = dram.tile([NCORES * 128], f32, addr_space="Shared")

            with tc.tile_pool(name="keep", bufs=1) as keep:
                t33all = keep.tile([128, NB], f32, name="t33all", tag="t33")
                sgbias = keep.tile([128, NB], f32, name="sgbias", tag="sgb")
                xr0 = keep.tile([128, N], f32r, name="xr0", tag="xr0")
                xr1 = keep.tile([128, N], f32r, name="xr1", tag="xr1")
                xg0 = keep.tile([128, RPC], f32r, name="xg0", tag="xg0")
                xg1 = keep.tile([128, RPC], f32r, name="xg1", tag="xg1")
                cb = keep.tile([128, N], f32, name="cb", tag="cb")

                # PE warmup: dummy matmuls to start the p-state ramp
                with tc.tile_pool(name="warm", bufs=1) as warm, \
                     tc.tile_pool(name="wps", bufs=1, space="PSUM") as wps:
                    wsf = warm.tile([128, 512], f32, name="wsf", tag="wf")
                    wsrc = warm.tile([128, 512], f32r, name="wsrc", tag="ws")
                    wp = wps.tile([128, 512], f32, name="wp", tag="wp")
                    nc.vector.memset(wsf[:], 0.0)
                    nc.scalar.activation(wsrc[:], wsf[:], COPY)
                    for _ in range(10):
                        nc.tensor.matmul(wp[:], wsrc[:, 0:128], wsrc[:],
                                         start=True, stop=True)

                # chunked input loads (first matmuls start early)
                nc.sync.dma_start(xg0[:], xgT[0:128, :])
                nc.sync.dma_start(xg1[:], xgT[128:256, :])
                bounds = [0, 256, 512, 1024, 2048, 3072, 4096, 6144, 8192]
                for c in range(len(bounds) - 1):
                    c0, c1 = bounds[c], bounds[c + 1]
                    nc.sync.dma_start(xr0[:, c0:c1], xT[0:128, c0:c1])
                    nc.sync.dma_start(xr1[:, c0:c1], xT[128:256, c0:c1])

                def mega_matmuls(ps_pool, rb, half, tag):
                    """Four [128,1024] psum tiles (= one half-block of 4096
                    cols); each stationary serves 8 consecutive MMs."""
                    r0, r1 = rb * 128, (rb + 1) * 128
                    base = half * 4096
                    mg = [ps_pool.tile([128, 1024], f32, name="mg", tag=tag)
                          for _ in range(4)]
                    for g in range(4):
                        for t in range(2):
                            c0 = base + g * 1024 + t * 512
                            nc.tensor.matmul(mg[g][:, t * 512:(t + 1) * 512],
                                             xg0[:, r0:r1],
                                             xr0[:, c0:c0 + 512],
                                             start=True, stop=False)
                    for g in range(4):
                        for t in range(2):
                            c0 = base + g * 1024 + t * 512
                            nc.tensor.matmul(mg[g][:, t * 512:(t + 1) * 512],
                                             xg1[:, r0:r1],
                                             xr1[:, c0:c0 + 512],
                                             start=False, stop=True)
                    return mg

                # ---------------- Phase 1: thresholds ----------------
                with tc.tile_pool(name="ps1", bufs=4, space="PSUM") as ps1, \
                     tc.tile_pool(name="thr", bufs=1) as thr:
                    cand = thr.tile([128, 256], f32, name="cand", tag="cand")
                    m8x = thr.tile([128, 17], f32, name="m8x", tag="m8x")
                    m8a, m8b, tmid = m8x[:, 0:8], m8x[:, 8:16], m8x[:, 16:17]
                    for rb in range(NB):
                        for half in range(2):
                            mg = mega_matmuls(ps1, rb, half, "p")
                            for g in range(4):
                                mi = half * 4 + g
                                for ch in range(4):
                                    o = mi * 32 + ch * 8
                                    nc.vector.max(
                                        out=cand[:, o:o + 8],
                                        in_=mg[g][:, ch * 256:(ch + 1) * 256])
                        for r in range(4):
                            nc.vector.max(out=m8a, in_=cand[:])
                            nc.vector.match_replace(out=cand[:],
                                                    in_to_replace=m8a,
                                                    in_values=cand[:],
                                                    imm_value=NEG)
                        nc.vector.max(out=m8b, in_=cand[:])
                        nc.vector.tensor_copy(t33all[:, rb:rb + 1],
                                              m8b[:, 0:1])
                        nc.vector.tensor_add(tmid, m8a[:, 7:8], m8b[:, 0:1])
                        nc.vector.tensor_scalar_mul(tmid, tmid, 0.5)
                        nc.vector.tensor_scalar_mul(
                            sgbias[:, rb:rb + 1], tmid, -float(SIGBIG))
                        if rb < 7:
                            nc.sync.dma_start(
                                t_loc_a[rb * 128:(rb + 1) * 128], tmid)
                        else:
                            nc.sync.dma_start(t_loc_b[0:128], tmid)
                        if rb == 6:
                            nc.gpsimd.collective_compute(
                                "AllGather", mybir.AluOpType.bypass,
                                replica_groups=[list(range(NCORES))],
                                ins=[t_loc_a.opt()], outs=[t_all_a.opt()])

                # ------------- AllGather part 2 (block 7 only) ----------
                nc.gpsimd.collective_compute(
                    "AllGather", mybir.AluOpType.bypass,
                    replica_groups=[list(range(NCORES))],
                    ins=[t_loc_b.opt()], outs=[t_all_b.opt()])

                # cb columns [c*1024, (c+1)*1024) belong to core c: first 896
                # from AG1 (blocks 0-6), last 128 from AG2 (block 7)
                for c in range(NCORES):
                    nc.sync.dma_start(
                        cb[:, c * 1024:c * 1024 + 896],
                        t_all_a.tensor.reshape([1, NCORES * 896])
                        .ap()[:, c * 896:(c + 1) * 896]
                        .to_broadcast((128, 896)))
                for c in range(NCORES):
                    nc.sync.dma_start(
                        cb[:, c * 1024 + 896:(c + 1) * 1024],
                        t_all_b.tensor.reshape([1, NCORES * 128])
                        .ap()[:, c * 128:(c + 1) * 128]
                        .to_broadcast((128, 128)))

                # ---------------- Phase 3: recompute + mask ----------------
                # Per mega-tile: ScalarE writes h5 = 5*psum (bf16, values
                # only); DVE does both compares on RAW psum (cc vs column
                # thresholds, then fused row-compare+add); Pool does the
                # final multiply h5*m -> bf16 output.
                with tc.tile_pool(name="ps3", bufs=4, space="PSUM") as ps3, \
                     tc.tile_pool(name="hs", bufs=6) as hsp, \
                     tc.tile_pool(name="mk", bufs=6) as mk, \
                     tc.tile_pool(name="ob", bufs=2) as obp:
                    for rb in range(NB):
                        r0, r1 = rb * 128, (rb + 1) * 128
                        for half in range(2):
                            mg = mega_matmuls(ps3, rb, half, "q")
                            ob = obp.tile([128, 4096], bf16, name="ob",
                                          tag="ob")
                            for g in range(4):
                                mi = half * 4 + g
                                c0 = mi * 1024
                                h5 = hsp.tile([128, 1024], bf16, name="h5",
                                              tag="h5")
                                nc.scalar.activation(h5[:], mg[g][:], COPY,
                                                     scale=float(SCALE))
                                # row mask as a saturated sigmoid (exact 0/1:
                                # tmid is strictly between e33 and e32)
                                ia = mk.tile([128, 1024], bf16, name="ia",
                                             tag="ia")
                                nc.scalar.activation(
                                    ia[:], mg[g][:], SIG,
                                    scale=float(SIGBIG),
                                    bias=sgbias[:, rb:rb + 1])
                                cc = mk.tile([128, 1024], bf16, name="cc",
                                             tag="cc")
                                nc.vector.tensor_tensor(
                                    out=cc[:, 0:896], in0=mg[g][:, 0:896],
                                    in1=cb[:, c0:c0 + 896], op=GT)
                                nc.vector.tensor_tensor(
                                    out=cc[:, 896:1024],
                                    in0=mg[g][:, 896:1024],
                                    in1=cb[:, c0 + 896:c0 + 1024], op=GT)
                                m = mk.tile([128, 1024], bf16, name="m",
                                            tag="m")
                                nc.vector.tensor_tensor(
                                    out=m[:, 0:896], in0=ia[:, 0:896],
                                    in1=cc[:, 0:896], op=ADD)
                                nc.vector.tensor_tensor(
                                    out=m[:, 896:1024], in0=ia[:, 896:1024],
                                    in1=cc[:, 896:1024], op=ADD)
                                if mi % 2 == 1:
                                    nc.vector.tensor_tensor(
                                        out=ob[:, g * 1024:(g + 1) * 1024],
                                        in0=h5[:], in1=m[:], op=MUL)
                                else:
                                    nc.gpsimd.tensor_tensor(
                                        out=ob[:, g * 1024:(g + 1) * 1024],
                                        in0=h5[:], in1=m[:], op=MUL)
                            g0 = half * 4096
                            nc.sync.dma_start(out[r0:r1, g0:g0 + 4096],
                                              ob[:])

    nc.compile()
    return nc


_nc_cache = None


def get_nc():
    global _nc_cache
    if _nc_cache is None:
        _nc_cache = build_nc()
    return _nc_cache


def kernel_with_result(x, trace: bool = False):
    x = np.ascontiguousarray(np.asarray(x), dtype=np.float32)
    assert x.shape == (N, DIM)
    nc = get_nc()
    xT = np.ascontiguousarray(x.T)
    in_maps = []
    for i in range(NCORES):
        xg = np.ascontiguousarray(x[i * RPC:(i + 1) * RPC, :].T)
        in_maps.append({"xT": xT, "xgT": xg})
    res = run_bass_kernel_spmd(nc, in_maps, core_ids=list(range(NCORES)),
                               trace=trace)
    outp = np.concatenate(
        [np.asarray(res.results[i]["out"]).astype(np.float32)
         for i in range(NCORES)], axis=0)
    return outp, res


def kernel(x) -> np.ndarray:
    outp, _res = kernel_with_result(x)
    return outp

